# revision 39
# baseline (speedup 1.0000x reference)
"""LocalRNN Trainium2 kernel: GLU -> pointwise conv -> 9-step windowed LSTM.

Full inputs in, full output out. Sharding: batch across 8 cores (2 batches/core).

Final design (vs the 322us bf16 baseline; measures ~255us, rel err 1.61e-2):
- fp16 pipeline everywhere bf16 was (same size/speed, 10 vs 8 mantissa bits):
  sim rel err 1.4e-3 vs 1.06e-2. The freed error budget pays for fp8.
- LSTM recurrence matmuls for steps 1-7 run in fp8e4 DoubleRow perf mode:
  each MM contracts 256 dims (2 packed/cell) at the same ~220ns FD=512
  cadence as bf16's 128 -> ~1.9x on the h-side PE stream. h written as
  fp8 by the cell's DVE mul; whh pre-packed [128,2,2048] pairs on host.
  Step 8 (error-critical, no forget-gate damping) stays fp16.
  Measured end-to-end rel err 1.611e-2, deterministic across runs.
- PSUM as 4 slots of [128,1024] (one per gate, both batches): a slot's
  fill (PE) and drain (one ACT op) serialize per slot, so big 2048-wide
  tiles bound the unit at fill+drain; 4 small slots pipeline instead.
- Per-gate inject bursts paired (I+G, F+O): batch-dependent gt rotation
  r=(c+q+2b)%4 (two wf copies) gives each 16-matmul burst 16 distinct
  PE (row_grp, col_grp) tiles -> fully concurrent injection.
- ACT order per unit: sigI, tanhG, sigF, sigO, tanhc feeds the DVE chain
  t1=sigI*tanhG, t2=sigF*c, c=t1+t2, h=sigO*tanh(c) without bubbles; each
  unit's tanh(c)+h write is deferred one unit so PSUM drains come earlier.
- Step boundaries: the first DR matmuls of a step wait on the previous
  step's last h8 write (tanh(c) at the end of the saturated ACT stream +
  the DVE fp8 mul). The j==0 unit hoists all four gate injects and every
  contraction-pair-0 MM (h from the early cells j0/j1) ahead of pair-1
  MMs, filling the stall with ~2.7us of runnable PE work so the HAM
  clock gate stops downgrading.
- Step 0 (h=0) skips the F gate entirely.
"""
from contextlib import ExitStack

import numpy as np
import ml_dtypes

import concourse.bass as bass
import concourse.mybir as mybir
import concourse.tile as tile
from concourse import bacc, bass_utils
from concourse.masks import make_identity

F32 = mybir.dt.float32
F16 = mybir.dt.float16
F8 = mybir.dt.float8e4
AF = mybir.ActivationFunctionType
DR = mybir.MatmulPerfMode.DoubleRow

N_CORES = 8
B_PER_CORE = 2          # batches per core
L = 512                 # sequence length
NT = B_PER_CORE * L     # tokens per core = 1024
D = 512                 # model dim
DH = 256                # GLU half dim
G4 = 4 * D              # 2048 gate rows
K = 9                   # window size
PAD = K - 1             # 8
LW = PAD + L            # 520: per-(tile,batch) padded gt row width
TW = B_PER_CORE * LW    # 1040: per-tile width in the big gt tile
S_FP8 = 7               # steps 1..S_FP8 use fp8 DoubleRow; rest fp16

_cache = {}


def _build():
    nc = bacc.Bacc(
        trn_type="TRN2", target_bir_lowering=False, debug=False, num_devices=N_CORES
    )

    xt_d = nc.dram_tensor("xt", [D, NT], F16, kind="ExternalInput").ap()
    wf_d = [nc.dram_tensor(f"wf{b}", [DH, G4], F16, kind="ExternalInput").ap()
            for b in range(2)]
    whh8_d = nc.dram_tensor("whh8", [128, 2 * 2 * G4], F8, kind="ExternalInput").ap()
    whh16_d = nc.dram_tensor("whh16", [D, G4], F16, kind="ExternalInput").ap()
    bias_d = nc.dram_tensor("bias", [128, 64], F32, kind="ExternalInput").ap()
    pad_d = nc.dram_tensor("gtpad", [128, 16 * 2 * PAD], F16, kind="ExternalInput").ap()
    out_d = nc.dram_tensor("out", [D, NT], F16, kind="ExternalOutput").ap()

    with tile.TileContext(nc) as tc, ExitStack() as top:
        const_pool = top.enter_context(tc.tile_pool(name="const", bufs=1))
        w_pool = top.enter_context(tc.tile_pool(name="weights", bufs=1))
        state_pool = top.enter_context(tc.tile_pool(name="state", bufs=1))

        z512 = const_pool.tile([128, 512], F16, tag="z512")
        nc.gpsimd.memset(z512[:], 0.0)
        ident = const_pool.tile([128, 128], F16, tag="id")
        make_identity(nc, ident[:])
        bias_sb = const_pool.tile([128, 64], F32, tag="bias")
        nc.scalar.dma_start(bias_sb[:], bias_d)

        xt = [w_pool.tile([128, NT], F16, tag=f"xt{t}", name=f"xt{t}")
              for t in range(4)]
        wf = [[w_pool.tile([128, G4], F16, tag=f"wf{b}_{ck}", name=f"wf{b}_{ck}")
               for ck in range(2)] for b in range(2)]
        whh8 = [w_pool.tile([128, 2 * G4], F8, tag=f"wh8_{p}", name=f"wh8_{p}")
                for p in range(2)]
        whh16 = [w_pool.tile([128, G4], F16, tag=f"wh16_{dk}", name=f"wh16_{dk}")
                 for dk in range(4)]
        # sync queue carries the big loads in need-order; tiny bias/pad on scalar
        for t in (2, 3, 0, 1):
            nc.sync.dma_start(xt[t][:], xt_d[t * 128:(t + 1) * 128, :])
        for b in range(2):
            for ck in range(2):
                nc.sync.dma_start(wf[b][ck][:], wf_d[b][ck * 128:(ck + 1) * 128, :])
        for p in range(2):
            nc.sync.dma_start(whh8[p][:], whh8_d[:, p * 2 * G4:(p + 1) * 2 * G4])
        for dk in range(4):
            nc.sync.dma_start(whh16[dk][:], whh16_d[dk * 128:(dk + 1) * 128, :])

        # big gt table: 16 gate-row tiles x (2 batches x 520), fp16
        gt = state_pool.tile([128, 16 * TW], F16, tag="gt", name="gt")
        # fp8 h pairs [gen][pair]: [128, 2, NT] -> DoubleRow moving operand
        h8 = [[state_pool.tile([128, 2 * NT], F8, tag=f"h8_{g}_{p}", name=f"h8_{g}_{p}")
               for p in range(2)] for g in range(2)]
        # fp16 h [gen][dk] for steps 7..8
        h16 = [[state_pool.tile([128, NT], F16, tag=f"h16_{g}_{j}", name=f"h16_{g}_{j}")
                for j in range(4)] for g in range(2)]
        cT = [state_pool.tile([128, NT], F16, tag=f"c{j}", name=f"c{j}") for j in range(4)]
        outT = [state_pool.tile([128, NT], F16, tag=f"o{j}", name=f"o{j}") for j in range(4)]
        uT = [state_pool.tile([128, NT], F16, tag=f"uT{ci}", name=f"uT{ci}")
              for ci in range(2)]

        h8v = [[h8[g][p][:].rearrange("p (k n) -> p k n", k=2) for p in range(2)]
               for g in range(2)]
        wh8v = [whh8[p][:].rearrange("p (k m) -> p k m", k=2) for p in range(2)]

        tp = top.enter_context(tc.tile_pool(name="tmp", bufs=4))
        psg = top.enter_context(tc.tile_pool(name="psg", bufs=4, space="PSUM"))

        def warm(n):
            """n dummy N=512 matmul pairs to trip/hold the HAM clock gate."""
            for _ in range(n):
                P = psg.tile([128, 1024], F32, tag="P", name="Pw")
                for r in range(4):
                    nc.tensor.matmul(
                        P[:, (r % 2) * 512:(r % 2) * 512 + 512], ident[:], z512[:],
                        start=True, stop=True,
                    )

        # pad columns (bias-only virtual tokens) land by DMA straight into
        # the strided pad slots of the big gt tile
        pad_view = gt[:].rearrange("p (i b c) -> p i b c", i=16, b=2)[:, :, :, 0:PAD]
        nc.scalar.dma_start(pad_view, pad_d)

        warm(6)  # cold-clock ramp; overlaps input DMA waits

        # GLU on transposed x: uT[ci] = xt[ci] * sigmoid(xt[2+ci])
        for ci in range(2):
            sgt = tp.tile([128, NT], F16, tag="tI", name="sgt")
            nc.scalar.activation(sgt[:], xt[2 + ci][:], AF.Sigmoid)
            nc.vector.tensor_mul(uT[ci][:], xt[ci][:], sgt[:])

        def g_phase(b, j):
            """gt tiles (q, j) for batch b: conv-folded input gates + bias."""
            for qp in range(2):
                P = psg.tile([128, 1024], F32, tag="P", name="Pg")
                for dq in range(2):
                    q = 2 * qp + dq
                    for ck in range(2):
                        nc.tensor.matmul(
                            P[:, dq * 512:(dq + 1) * 512],
                            wf[b][ck][:, (4 * q + j) * 128:(4 * q + j + 1) * 128],
                            uT[ck][:, b * 512:(b + 1) * 512],
                            start=(ck == 0), stop=(ck == 1),
                        )
                for dq in range(2):
                    q = 2 * qp + dq
                    i = 4 * q + j
                    dst = gt[:, i * TW + b * LW + PAD:i * TW + b * LW + LW]
                    bcol = bias_sb[:, b * 32 + i:b * 32 + i + 1]
                    if q <= 1:
                        nc.vector.tensor_scalar_add(dst, P[:, dq * 512:(dq + 1) * 512], bcol)
                    else:
                        nc.scalar.activation(
                            dst, P[:, dq * 512:(dq + 1) * 512], AF.Identity, bias=bcol,
                        )

        def inject(P, j, q, k, stop):
            """gt -> one [128,1024] gate slot (both batches): 8 32x32 tile MMs.
            Batch-dependent rotation r=(c+q+2b)%4 keeps every paired burst of
            two gates (I+G or F+O) on 16 distinct PE array tiles."""
            for b in range(2):
                off = (4 * q + j) * TW + b * LW + k
                for c in range(4):
                    r = (c + q + 2 * b) % 4
                    nc.tensor.matmul(
                        P[32 * c:32 * c + 32, b * 512:b * 512 + 512],
                        ident[32 * r:32 * r + 32, 32 * r:32 * r + 32],
                        gt[32 * r:32 * r + 32, off:off + 512],
                        start=True, stop=stop,
                        tile_position=(32 * r, 32 * c),
                    )

        def dr8(tiles, j, gr, ponly=None):
            """fp8 DoubleRow recurrence MMs into a list of (P, q) gate slots.
            Contraction-pair-outer order: the pair-0 MMs (h from cells j0/j1,
            written early in the previous step) of every slot run before any
            pair-1 MM needs the last cell's h -> the step-boundary stall on
            the final h8 write shrinks by ~2us of runnable PE work."""
            prange = range(2) if ponly is None else [ponly]
            for p in prange:
                for P, q in tiles:
                    for b in range(2):
                        nc.tensor.matmul(
                            P[:, b * 512:(b + 1) * 512],
                            wh8v[p][:, :, (4 * q + j) * 128:(4 * q + j + 1) * 128],
                            h8v[gr][p][:, :, b * 512:(b + 1) * 512],
                            start=False, stop=(p == 1), perf_mode=DR,
                        )

        def mm16(tiles, j, gr):
            """fp16 recurrence MMs, dk-outer for the same reason as dr8."""
            for dk in range(4):
                for P, q in tiles:
                    for b in range(2):
                        nc.tensor.matmul(
                            P[:, b * 512:(b + 1) * 512],
                            whh16[dk][:, (4 * q + j) * 128:(4 * q + j + 1) * 128],
                            h16[gr][dk][:, b * 512:(b + 1) * 512],
                            start=False, stop=(dk == 3),
                        )

        def cell_front(k, j, PI, PF, PO, PG):
            """gate drains + c update; returns sig(O) for cell_back."""
            first = k == 0
            tI = tp.tile([128, 1024], F16, tag="tI", name="tI")
            nc.scalar.activation(tI[:], PI, AF.Sigmoid)
            tG = tp.tile([128, 1024], F16, tag="tG", name="tG")
            nc.scalar.activation(tG[:], PG, AF.Tanh)
            if first:
                nc.vector.tensor_mul(cT[j][:], tI[:], tG[:])
            else:
                t1 = tp.tile([128, 1024], F16, tag="t1", name="t1")
                nc.vector.tensor_mul(t1[:], tI[:], tG[:])
                tF = tp.tile([128, 1024], F16, tag="tF", name="tF")
                nc.scalar.activation(tF[:], PF, AF.Sigmoid)
                t2 = tp.tile([128, 1024], F16, tag="tG", name="t2")
                nc.vector.tensor_mul(t2[:], tF[:], cT[j][:])
                nc.vector.tensor_add(cT[j][:], t1[:], t2[:])
            tO = tp.tile([128, 1024], F16, tag="tO", name="tO")
            nc.scalar.activation(tO[:], PO, AF.Sigmoid)
            return tO

        def back_half(k, j, tO, tTc, sl):
            nc.scalar.activation(tTc[:, sl], cT[j][:, sl], AF.Tanh)
            if k == K - 1:
                nc.vector.tensor_mul(outT[j][:, sl], tO[:, sl], tTc[:, sl])
            elif k >= S_FP8:
                nc.vector.tensor_mul(h16[k % 2][j][:, sl], tO[:, sl], tTc[:, sl])
            else:
                dst = h8v[k % 2][j // 2][:, j % 2, sl]
                nc.vector.tensor_mul(dst, tO[:, sl], tTc[:, sl])

        def cell_back(k, j, tO):
            """tanh(c) + h write, deferred one unit so the next unit's PSUM
            drains run earlier on ACT."""
            tTc = tp.tile([128, 1024], F16, tag="tTc", name="tTc")
            back_half(k, j, tO, tTc, slice(0, 1024))
            if k == K - 1:
                eng = nc.sync if j % 2 == 0 else nc.scalar
                eng.dma_start(out_d[j * 128:(j + 1) * 128, :], outT[j][:])

        def cell_back_split(k, j, tO):
            """step's last unit: per-batch halves so the first half of the
            next step's h dependency resolves earlier."""
            tTc = tp.tile([128, 1024], F16, tag="tTc", name="tTc")
            back_half(k, j, tO, tTc, slice(0, 512))
            back_half(k, j, tO, tTc, slice(512, 1024))
            if k == K - 1:
                eng = nc.sync if j % 2 == 0 else nc.scalar
                eng.dma_start(out_d[j * 128:(j + 1) * 128, :], outT[j][:])

        def unit0(j):
            """step 0 (h=0): no recurrence, no F gate."""
            PI = psg.tile([128, 1024], F32, tag="P", name="P0I")
            PG = psg.tile([128, 1024], F32, tag="P", name="P0G")
            inject(PI, j, 0, 0, stop=True)
            inject(PG, j, 3, 0, stop=True)
            PO = psg.tile([128, 1024], F32, tag="P", name="P0O")
            inject(PO, j, 2, 0, stop=True)
            tO0 = cell_front(0, j, PI[:], None, PO[:], PG[:])
            cell_back(0, j, tO0)

        def unit(k, j):
            gr = (k + 1) % 2
            mms = dr8 if k <= S_FP8 else mm16
            PI = psg.tile([128, 1024], F32, tag="P", name="PI")
            PG = psg.tile([128, 1024], F32, tag="P", name="PG")
            inject(PI, j, 0, k, stop=False)
            inject(PG, j, 3, k, stop=False)
            if j == 0 and k <= S_FP8:
                # step boundary: PE idles ~2.5us on the last h8 write (pair 1).
                # Hoist the F/O injects and every pair-0 MM ahead of the
                # pair-1 MMs: ~2.7us of h8-independent work fills the stall.
                # ACT's first drain waits on the pair-1 stop either way.
                dr8([(PI, 0), (PG, 3)], j, gr, ponly=0)
                PF = psg.tile([128, 1024], F32, tag="P", name="PF")
                PO = psg.tile([128, 1024], F32, tag="P", name="PO")
                inject(PF, j, 1, k, stop=False)
                inject(PO, j, 2, k, stop=False)
                dr8([(PF, 1), (PO, 2)], j, gr, ponly=0)
                dr8([(PI, 0), (PG, 3)], j, gr, ponly=1)
                dr8([(PF, 1), (PO, 2)], j, gr, ponly=1)
            else:
                mms([(PI, 0), (PG, 3)], j, gr)
                PF = psg.tile([128, 1024], F32, tag="P", name="PF")
                PO = psg.tile([128, 1024], F32, tag="P", name="PO")
                inject(PF, j, 1, k, stop=False)
                inject(PO, j, 2, k, stop=False)
                mms([(PF, 1), (PO, 2)], j, gr)
            return cell_front(k, j, PI[:], PF[:], PO[:], PG[:])

        # prep: per-j g_phase for both batches, then step-0 cell
        for j in range(4):
            g_phase(0, j)
            g_phase(1, j)
            unit0(j)
        warm(2)  # filler across the prep -> steady-state transition

        # ---------------- LSTM steps 1..8 ----------------
        for k in range(1, K):
            pend = None
            for j in range(4):
                tO = unit(k, j)
                if pend is not None:
                    cell_back(k, pend[0], pend[1])
                pend = (j, tO)
            if k < K - 1:
                cell_back_split(k, 3, pend[1])
            else:
                cell_back(k, 3, pend[1])

    nc.compile()
    return nc


def _make_in_maps(inputs):
    x = np.asarray(inputs["x"], dtype=np.float32)
    conv_w = np.asarray(inputs["conv_w"], dtype=np.float64)
    conv_b = np.asarray(inputs["conv_b"], dtype=np.float64)
    w_ih = np.asarray(inputs["w_ih"], dtype=np.float64)
    w_hh = np.asarray(inputs["w_hh"], dtype=np.float32)
    b_ih = np.asarray(inputs["b_ih"], dtype=np.float64)
    b_hh = np.asarray(inputs["b_hh"], dtype=np.float64)

    # gate permutation: torch order i,f,g,o -> i,f,o,g
    perm = np.concatenate([
        np.arange(0, D), np.arange(D, 2 * D),
        np.arange(3 * D, 4 * D), np.arange(2 * D, 3 * D),
    ])
    wf_p = (w_ih @ conv_w)[perm]                                # [2048, 256]
    bias_mm = (b_ih + b_hh + w_ih @ conv_b)[perm]
    bias_pad = (b_ih + b_hh)[perm]

    # batch-dependent rotation: tile i (=4q+j) stores logical 32-row block c
    # at partition block r=(c+q+2b)%4 for batch b's gt columns
    def make_rot(b):
        rot = np.empty(G4, dtype=np.int64)
        for i in range(16):
            q = i // 4
            for c in range(4):
                r = (c + q + 2 * b) % 4
                rot[i * 128 + 32 * r: i * 128 + 32 * r + 32] = \
                    np.arange(i * 128 + 32 * c, i * 128 + 32 * c + 32)
        return rot

    shared = {}
    bias_both = np.empty((128, 64), np.float32)
    gtpad = np.empty((128, 16, 2, PAD), np.float16)
    for b in range(2):
        rot = make_rot(b)
        shared[f"wf{b}"] = np.ascontiguousarray(
            wf_p[rot].T.astype(np.float16))                     # [256, 2048]
        bias_both[:, b * 32:b * 32 + 16] = \
            bias_mm[rot].astype(np.float32).reshape(16, 128).T
        bias_both[:, b * 32 + 16:b * 32 + 32] = \
            bias_pad[rot].astype(np.float32).reshape(16, 128).T
        bp = bias_pad[rot].astype(np.float16).reshape(16, 128).T      # [128,16]
        gtpad[:, :, b, :] = bp[:, :, None]
    shared["bias"] = bias_both
    shared["gtpad"] = np.ascontiguousarray(gtpad.reshape(128, -1))

    whhT = np.ascontiguousarray(w_hh[perm].T)                   # [512, 2048]
    shared["whh16"] = whhT.astype(np.float16)
    blocks = whhT.reshape(4, 128, G4)
    pairs = np.stack([np.stack([blocks[2 * p], blocks[2 * p + 1]], axis=1)
                      for p in range(2)])                       # [2, 128, 2, 2048]
    shared["whh8"] = np.ascontiguousarray(
        pairs.transpose(1, 0, 2, 3).reshape(128, -1).astype(ml_dtypes.float8_e4m3fn))

    in_maps = []
    for c in range(N_CORES):
        m = dict(shared)
        xc = x[c * B_PER_CORE:(c + 1) * B_PER_CORE].reshape(NT, D)
        m["xt"] = np.ascontiguousarray(xc.T.astype(np.float16))  # [512, 1024]
        in_maps.append(m)
    return in_maps


def kernel(x, conv_w, conv_b, w_ih, w_hh, b_ih, b_hh):
    if "nc" not in _cache:
        _cache["nc"] = _build()
    nc = _cache["nc"]

    in_maps = _make_in_maps(dict(
        x=x, conv_w=conv_w, conv_b=conv_b, w_ih=w_ih, w_hh=w_hh,
        b_ih=b_ih, b_hh=b_hh,
    ))

    res = bass_utils.run_bass_kernel_spmd(nc, in_maps, core_ids=list(range(N_CORES)))
    out = np.concatenate(
        [np.ascontiguousarray(np.asarray(r["out"]).astype(np.float32).T)
         .reshape(B_PER_CORE, L, D)
         for r in res.results], axis=0
    )
    return out


# revision 40
# speedup vs baseline: 1.0017x; 1.0017x over previous
"""LocalRNN Trainium2 kernel: GLU -> pointwise conv -> 9-step windowed LSTM.

Full inputs in, full output out. Sharding: batch across 8 cores (2 batches/core).

Final design (vs the 322us bf16 baseline; measures ~255us, rel err 1.61e-2):
- fp16 pipeline everywhere bf16 was (same size/speed, 10 vs 8 mantissa bits):
  sim rel err 1.4e-3 vs 1.06e-2. The freed error budget pays for fp8.
- LSTM recurrence matmuls for steps 1-7 run in fp8e4 DoubleRow perf mode:
  each MM contracts 256 dims (2 packed/cell) at the same ~220ns FD=512
  cadence as bf16's 128 -> ~1.9x on the h-side PE stream. h written as
  fp8 by the cell's DVE mul; whh pre-packed [128,2,2048] pairs on host.
  Step 8 (error-critical, no forget-gate damping) stays fp16.
  Measured end-to-end rel err 1.611e-2, deterministic across runs.
- PSUM as 4 slots of [128,1024] (one per gate, both batches): a slot's
  fill (PE) and drain (one ACT op) serialize per slot, so big 2048-wide
  tiles bound the unit at fill+drain; 4 small slots pipeline instead.
- Per-gate inject bursts paired (I+G, F+O): batch-dependent gt rotation
  r=(c+q+2b)%4 (two wf copies) gives each 16-matmul burst 16 distinct
  PE (row_grp, col_grp) tiles -> fully concurrent injection.
- ACT order per unit: sigI, tanhG, sigF, sigO, tanhc feeds the DVE chain
  t1=sigI*tanhG, t2=sigF*c, c=t1+t2, h=sigO*tanh(c) without bubbles; each
  unit's tanh(c)+h write is deferred one unit so PSUM drains come earlier.
- Step boundaries: the first DR matmuls of a step wait on the previous
  step's last h8 write (tanh(c) at the end of the saturated ACT stream +
  the DVE fp8 mul). The j==0 unit hoists all four gate injects and every
  contraction-pair-0 MM (h from the early cells j0/j1) ahead of pair-1
  MMs, filling the stall with ~2.7us of runnable PE work so the HAM
  clock gate stops downgrading.
- Step 0 (h=0) skips the F gate entirely.
"""
from contextlib import ExitStack

import numpy as np
import ml_dtypes

import concourse.bass as bass
import concourse.mybir as mybir
import concourse.tile as tile
from concourse import bacc, bass_utils
from concourse.masks import make_identity

F32 = mybir.dt.float32
F16 = mybir.dt.float16
F8 = mybir.dt.float8e4
AF = mybir.ActivationFunctionType
DR = mybir.MatmulPerfMode.DoubleRow

N_CORES = 8
B_PER_CORE = 2          # batches per core
L = 512                 # sequence length
NT = B_PER_CORE * L     # tokens per core = 1024
D = 512                 # model dim
DH = 256                # GLU half dim
G4 = 4 * D              # 2048 gate rows
K = 9                   # window size
PAD = K - 1             # 8
LW = PAD + L            # 520: per-(tile,batch) padded gt row width
TW = B_PER_CORE * LW    # 1040: per-tile width in the big gt tile
S_FP8 = 7               # steps 1..S_FP8 use fp8 DoubleRow; rest fp16

_cache = {}


def _build():
    nc = bacc.Bacc(
        trn_type="TRN2", target_bir_lowering=False, debug=False, num_devices=N_CORES
    )

    xt_d = nc.dram_tensor("xt", [D, NT], F16, kind="ExternalInput").ap()
    wf_d = [nc.dram_tensor(f"wf{b}", [DH, G4], F16, kind="ExternalInput").ap()
            for b in range(2)]
    whh8_d = nc.dram_tensor("whh8", [128, 2 * 2 * G4], F8, kind="ExternalInput").ap()
    whh16_d = nc.dram_tensor("whh16", [D, G4], F16, kind="ExternalInput").ap()
    bias_d = nc.dram_tensor("bias", [128, 64], F32, kind="ExternalInput").ap()
    pad_d = nc.dram_tensor("gtpad", [128, 16 * 2 * PAD], F16, kind="ExternalInput").ap()
    out_d = nc.dram_tensor("out", [D, NT], F16, kind="ExternalOutput").ap()

    with tile.TileContext(nc) as tc, ExitStack() as top:
        const_pool = top.enter_context(tc.tile_pool(name="const", bufs=1))
        w_pool = top.enter_context(tc.tile_pool(name="weights", bufs=1))
        state_pool = top.enter_context(tc.tile_pool(name="state", bufs=1))

        z512 = const_pool.tile([128, 512], F16, tag="z512")
        nc.gpsimd.memset(z512[:], 0.0)
        ident = const_pool.tile([128, 128], F16, tag="id")
        make_identity(nc, ident[:])
        bias_sb = const_pool.tile([128, 64], F32, tag="bias")
        nc.scalar.dma_start(bias_sb[:], bias_d)

        xt = [w_pool.tile([128, NT], F16, tag=f"xt{t}", name=f"xt{t}")
              for t in range(4)]
        wf = [[w_pool.tile([128, G4], F16, tag=f"wf{b}_{ck}", name=f"wf{b}_{ck}")
               for ck in range(2)] for b in range(2)]
        whh8 = [w_pool.tile([128, 2 * G4], F8, tag=f"wh8_{p}", name=f"wh8_{p}")
                for p in range(2)]
        whh16 = [w_pool.tile([128, G4], F16, tag=f"wh16_{dk}", name=f"wh16_{dk}")
                 for dk in range(4)]
        # sync queue carries the big loads in need-order; tiny bias/pad on scalar
        for t in (2, 3, 0, 1):
            nc.sync.dma_start(xt[t][:], xt_d[t * 128:(t + 1) * 128, :])
        for b in range(2):
            for ck in range(2):
                nc.sync.dma_start(wf[b][ck][:], wf_d[b][ck * 128:(ck + 1) * 128, :])
        for p in range(2):
            nc.sync.dma_start(whh8[p][:], whh8_d[:, p * 2 * G4:(p + 1) * 2 * G4])
        for dk in range(4):
            nc.sync.dma_start(whh16[dk][:], whh16_d[dk * 128:(dk + 1) * 128, :])

        # big gt table: 16 gate-row tiles x (2 batches x 520), fp16
        gt = state_pool.tile([128, 16 * TW], F16, tag="gt", name="gt")
        # fp8 h pairs [gen][pair]: [128, 2, NT] -> DoubleRow moving operand
        h8 = [[state_pool.tile([128, 2 * NT], F8, tag=f"h8_{g}_{p}", name=f"h8_{g}_{p}")
               for p in range(2)] for g in range(2)]
        # fp16 h [gen][dk] for steps 7..8
        h16 = [[state_pool.tile([128, NT], F16, tag=f"h16_{g}_{j}", name=f"h16_{g}_{j}")
                for j in range(4)] for g in range(2)]
        cTall = state_pool.tile([128, 4 * NT], F16, tag="c", name="cAll")
        cT = [cTall[:][:, j * NT:(j + 1) * NT] for j in range(4)]
        outT = [state_pool.tile([128, NT], F16, tag=f"o{j}", name=f"o{j}") for j in range(4)]
        uT = [state_pool.tile([128, NT], F16, tag=f"uT{ci}", name=f"uT{ci}")
              for ci in range(2)]

        h8v = [[h8[g][p][:].rearrange("p (k n) -> p k n", k=2) for p in range(2)]
               for g in range(2)]
        wh8v = [whh8[p][:].rearrange("p (k m) -> p k m", k=2) for p in range(2)]

        tp = top.enter_context(tc.tile_pool(name="tmp", bufs=4))
        psg = top.enter_context(tc.tile_pool(name="psg", bufs=4, space="PSUM"))

        def warm(n):
            """n dummy N=512 matmul pairs to trip/hold the HAM clock gate."""
            for _ in range(n):
                P = psg.tile([128, 1024], F32, tag="P", name="Pw")
                for r in range(4):
                    nc.tensor.matmul(
                        P[:, (r % 2) * 512:(r % 2) * 512 + 512], ident[:], z512[:],
                        start=True, stop=True,
                    )

        # pad columns (bias-only virtual tokens) land by DMA straight into
        # the strided pad slots of the big gt tile
        pad_view = gt[:].rearrange("p (i b c) -> p i b c", i=16, b=2)[:, :, :, 0:PAD]
        nc.scalar.dma_start(pad_view, pad_d)

        warm(6)  # cold-clock ramp; overlaps input DMA waits

        # GLU on transposed x: uT[ci] = xt[ci] * sigmoid(xt[2+ci])
        for ci in range(2):
            sgt = tp.tile([128, NT], F16, tag="tI", name="sgt")
            nc.scalar.activation(sgt[:], xt[2 + ci][:], AF.Sigmoid)
            nc.vector.tensor_mul(uT[ci][:], xt[ci][:], sgt[:])

        def g_phase(b, j):
            """gt tiles (q, j) for batch b: conv-folded input gates + bias."""
            for qp in range(2):
                P = psg.tile([128, 1024], F32, tag="P", name="Pg")
                for dq in range(2):
                    q = 2 * qp + dq
                    for ck in range(2):
                        nc.tensor.matmul(
                            P[:, dq * 512:(dq + 1) * 512],
                            wf[b][ck][:, (4 * q + j) * 128:(4 * q + j + 1) * 128],
                            uT[ck][:, b * 512:(b + 1) * 512],
                            start=(ck == 0), stop=(ck == 1),
                        )
                for dq in range(2):
                    q = 2 * qp + dq
                    i = 4 * q + j
                    dst = gt[:, i * TW + b * LW + PAD:i * TW + b * LW + LW]
                    bcol = bias_sb[:, b * 32 + i:b * 32 + i + 1]
                    if q <= 1:
                        nc.vector.tensor_scalar_add(dst, P[:, dq * 512:(dq + 1) * 512], bcol)
                    else:
                        nc.scalar.activation(
                            dst, P[:, dq * 512:(dq + 1) * 512], AF.Identity, bias=bcol,
                        )

        def inject(P, j, q, k, stop):
            """gt -> one [128,1024] gate slot (both batches): 8 32x32 tile MMs.
            Batch-dependent rotation r=(c+q+2b)%4 keeps every paired burst of
            two gates (I+G or F+O) on 16 distinct PE array tiles."""
            for b in range(2):
                off = (4 * q + j) * TW + b * LW + k
                for c in range(4):
                    r = (c + q + 2 * b) % 4
                    nc.tensor.matmul(
                        P[32 * c:32 * c + 32, b * 512:b * 512 + 512],
                        ident[32 * r:32 * r + 32, 32 * r:32 * r + 32],
                        gt[32 * r:32 * r + 32, off:off + 512],
                        start=True, stop=stop,
                        tile_position=(32 * r, 32 * c),
                    )

        def dr8(tiles, j, gr, ponly=None):
            """fp8 DoubleRow recurrence MMs into a list of (P, q) gate slots.
            Contraction-pair-outer order: the pair-0 MMs (h from cells j0/j1,
            written early in the previous step) of every slot run before any
            pair-1 MM needs the last cell's h -> the step-boundary stall on
            the final h8 write shrinks by ~2us of runnable PE work."""
            prange = range(2) if ponly is None else [ponly]
            for p in prange:
                for P, q in tiles:
                    for b in range(2):
                        nc.tensor.matmul(
                            P[:, b * 512:(b + 1) * 512],
                            wh8v[p][:, :, (4 * q + j) * 128:(4 * q + j + 1) * 128],
                            h8v[gr][p][:, :, b * 512:(b + 1) * 512],
                            start=False, stop=(p == 1), perf_mode=DR,
                        )

        def mm16(tiles, j, gr):
            """fp16 recurrence MMs, dk-outer for the same reason as dr8."""
            for dk in range(4):
                for P, q in tiles:
                    for b in range(2):
                        nc.tensor.matmul(
                            P[:, b * 512:(b + 1) * 512],
                            whh16[dk][:, (4 * q + j) * 128:(4 * q + j + 1) * 128],
                            h16[gr][dk][:, b * 512:(b + 1) * 512],
                            start=False, stop=(dk == 3),
                        )

        def cell_front(k, j, PI, PF, PO, PG):
            """gate drains + c update; returns sig(O) for cell_back."""
            first = k == 0
            tI = tp.tile([128, 1024], F16, tag="tI", name="tI")
            nc.scalar.activation(tI[:], PI, AF.Sigmoid)
            tG = tp.tile([128, 1024], F16, tag="tG", name="tG")
            nc.scalar.activation(tG[:], PG, AF.Tanh)
            if first:
                nc.vector.tensor_mul(cT[j], tI[:], tG[:])
            else:
                t1 = tp.tile([128, 1024], F16, tag="t1", name="t1")
                nc.vector.tensor_mul(t1[:], tI[:], tG[:])
                tF = tp.tile([128, 1024], F16, tag="tF", name="tF")
                nc.scalar.activation(tF[:], PF, AF.Sigmoid)
                t2 = tp.tile([128, 1024], F16, tag="tG", name="t2")
                nc.vector.tensor_mul(t2[:], tF[:], cT[j])
                nc.vector.tensor_add(cT[j], t1[:], t2[:])
            tO = tp.tile([128, 1024], F16, tag="tO", name="tO")
            nc.scalar.activation(tO[:], PO, AF.Sigmoid)
            return tO

        def back_half(k, j, tO, tTc, sl):
            nc.scalar.activation(tTc[:, sl], cT[j][:, sl], AF.Tanh)
            if k == K - 1:
                nc.vector.tensor_mul(outT[j][:, sl], tO[:, sl], tTc[:, sl])
            elif k >= S_FP8:
                nc.vector.tensor_mul(h16[k % 2][j][:, sl], tO[:, sl], tTc[:, sl])
            else:
                dst = h8v[k % 2][j // 2][:, j % 2, sl]
                nc.vector.tensor_mul(dst, tO[:, sl], tTc[:, sl])

        def cell_back(k, j, tO):
            """tanh(c) + h write, deferred one unit so the next unit's PSUM
            drains run earlier on ACT."""
            tTc = tp.tile([128, 1024], F16, tag="tTc", name="tTc")
            back_half(k, j, tO, tTc, slice(0, 1024))
            if k == K - 1:
                eng = nc.sync if j % 2 == 0 else nc.scalar
                eng.dma_start(out_d[j * 128:(j + 1) * 128, :], outT[j][:])

        def cell_back_pair(k, tO0, tO1):
            """merged deferred tails of units j0+j1: one 2048-wide tanh over
            the contiguous c slices saves an ACT op overhead per step."""
            tT2 = tp.tile([128, 2048], F16, tag="tT2", name="tT2")
            nc.scalar.activation(tT2[:], cTall[:][:, 0:2 * NT], AF.Tanh)
            for j, tO in ((0, tO0), (1, tO1)):
                sl2 = slice(j * 1024, (j + 1) * 1024)
                if k == K - 1:
                    nc.vector.tensor_mul(outT[j][:], tO[:], tT2[:, sl2])
                    eng = nc.sync if j % 2 == 0 else nc.scalar
                    eng.dma_start(out_d[j * 128:(j + 1) * 128, :], outT[j][:])
                elif k >= S_FP8:
                    nc.vector.tensor_mul(h16[k % 2][j][:], tO[:], tT2[:, sl2])
                else:
                    dst = h8v[k % 2][j // 2][:, j % 2, :]
                    nc.vector.tensor_mul(dst, tO[:], tT2[:, sl2])

        def cell_back_split(k, j, tO):
            """step's last unit: per-batch halves so the first half of the
            next step's h dependency resolves earlier."""
            tTc = tp.tile([128, 1024], F16, tag="tTc", name="tTc")
            back_half(k, j, tO, tTc, slice(0, 512))
            back_half(k, j, tO, tTc, slice(512, 1024))
            if k == K - 1:
                eng = nc.sync if j % 2 == 0 else nc.scalar
                eng.dma_start(out_d[j * 128:(j + 1) * 128, :], outT[j][:])

        def unit0(j):
            """step 0 (h=0): no recurrence, no F gate."""
            PI = psg.tile([128, 1024], F32, tag="P", name="P0I")
            PG = psg.tile([128, 1024], F32, tag="P", name="P0G")
            inject(PI, j, 0, 0, stop=True)
            inject(PG, j, 3, 0, stop=True)
            PO = psg.tile([128, 1024], F32, tag="P", name="P0O")
            inject(PO, j, 2, 0, stop=True)
            tO0 = cell_front(0, j, PI[:], None, PO[:], PG[:])
            cell_back(0, j, tO0)

        def unit(k, j):
            gr = (k + 1) % 2
            mms = dr8 if k <= S_FP8 else mm16
            PI = psg.tile([128, 1024], F32, tag="P", name="PI")
            PG = psg.tile([128, 1024], F32, tag="P", name="PG")
            inject(PI, j, 0, k, stop=False)
            inject(PG, j, 3, k, stop=False)
            if j == 0 and k <= S_FP8:
                # step boundary: PE idles ~2.5us on the last h8 write (pair 1).
                # Hoist the F/O injects and every pair-0 MM ahead of the
                # pair-1 MMs: ~2.7us of h8-independent work fills the stall.
                # ACT's first drain waits on the pair-1 stop either way.
                dr8([(PI, 0), (PG, 3)], j, gr, ponly=0)
                PF = psg.tile([128, 1024], F32, tag="P", name="PF")
                PO = psg.tile([128, 1024], F32, tag="P", name="PO")
                inject(PF, j, 1, k, stop=False)
                inject(PO, j, 2, k, stop=False)
                dr8([(PF, 1), (PO, 2)], j, gr, ponly=0)
                dr8([(PI, 0), (PG, 3)], j, gr, ponly=1)
                dr8([(PF, 1), (PO, 2)], j, gr, ponly=1)
            else:
                mms([(PI, 0), (PG, 3)], j, gr)
                PF = psg.tile([128, 1024], F32, tag="P", name="PF")
                PO = psg.tile([128, 1024], F32, tag="P", name="PO")
                inject(PF, j, 1, k, stop=False)
                inject(PO, j, 2, k, stop=False)
                mms([(PF, 1), (PO, 2)], j, gr)
            return cell_front(k, j, PI[:], PF[:], PO[:], PG[:])

        # prep: per-j g_phase for both batches, then step-0 cell
        for j in range(4):
            g_phase(0, j)
            g_phase(1, j)
            unit0(j)
        warm(2)  # filler across the prep -> steady-state transition

        # ---------------- LSTM steps 1..8 ----------------
        for k in range(1, K):
            tOs = []
            for j in range(4):
                tOs.append(unit(k, j))
                if j == 2:
                    cell_back_pair(k, tOs[0], tOs[1])
            cell_back(k, 2, tOs[2])
            if k < K - 1:
                cell_back_split(k, 3, tOs[3])
            else:
                cell_back(k, 3, tOs[3])

    nc.compile()
    return nc


def _make_in_maps(inputs):
    x = np.asarray(inputs["x"], dtype=np.float32)
    conv_w = np.asarray(inputs["conv_w"], dtype=np.float64)
    conv_b = np.asarray(inputs["conv_b"], dtype=np.float64)
    w_ih = np.asarray(inputs["w_ih"], dtype=np.float64)
    w_hh = np.asarray(inputs["w_hh"], dtype=np.float32)
    b_ih = np.asarray(inputs["b_ih"], dtype=np.float64)
    b_hh = np.asarray(inputs["b_hh"], dtype=np.float64)

    # gate permutation: torch order i,f,g,o -> i,f,o,g
    perm = np.concatenate([
        np.arange(0, D), np.arange(D, 2 * D),
        np.arange(3 * D, 4 * D), np.arange(2 * D, 3 * D),
    ])
    wf_p = (w_ih @ conv_w)[perm]                                # [2048, 256]
    bias_mm = (b_ih + b_hh + w_ih @ conv_b)[perm]
    bias_pad = (b_ih + b_hh)[perm]

    # batch-dependent rotation: tile i (=4q+j) stores logical 32-row block c
    # at partition block r=(c+q+2b)%4 for batch b's gt columns
    def make_rot(b):
        rot = np.empty(G4, dtype=np.int64)
        for i in range(16):
            q = i // 4
            for c in range(4):
                r = (c + q + 2 * b) % 4
                rot[i * 128 + 32 * r: i * 128 + 32 * r + 32] = \
                    np.arange(i * 128 + 32 * c, i * 128 + 32 * c + 32)
        return rot

    shared = {}
    bias_both = np.empty((128, 64), np.float32)
    gtpad = np.empty((128, 16, 2, PAD), np.float16)
    for b in range(2):
        rot = make_rot(b)
        shared[f"wf{b}"] = np.ascontiguousarray(
            wf_p[rot].T.astype(np.float16))                     # [256, 2048]
        bias_both[:, b * 32:b * 32 + 16] = \
            bias_mm[rot].astype(np.float32).reshape(16, 128).T
        bias_both[:, b * 32 + 16:b * 32 + 32] = \
            bias_pad[rot].astype(np.float32).reshape(16, 128).T
        bp = bias_pad[rot].astype(np.float16).reshape(16, 128).T      # [128,16]
        gtpad[:, :, b, :] = bp[:, :, None]
    shared["bias"] = bias_both
    shared["gtpad"] = np.ascontiguousarray(gtpad.reshape(128, -1))

    whhT = np.ascontiguousarray(w_hh[perm].T)                   # [512, 2048]
    shared["whh16"] = whhT.astype(np.float16)
    blocks = whhT.reshape(4, 128, G4)
    pairs = np.stack([np.stack([blocks[2 * p], blocks[2 * p + 1]], axis=1)
                      for p in range(2)])                       # [2, 128, 2, 2048]
    shared["whh8"] = np.ascontiguousarray(
        pairs.transpose(1, 0, 2, 3).reshape(128, -1).astype(ml_dtypes.float8_e4m3fn))

    in_maps = []
    for c in range(N_CORES):
        m = dict(shared)
        xc = x[c * B_PER_CORE:(c + 1) * B_PER_CORE].reshape(NT, D)
        m["xt"] = np.ascontiguousarray(xc.T.astype(np.float16))  # [512, 1024]
        in_maps.append(m)
    return in_maps


def kernel(x, conv_w, conv_b, w_ih, w_hh, b_ih, b_hh):
    if "nc" not in _cache:
        _cache["nc"] = _build()
    nc = _cache["nc"]

    in_maps = _make_in_maps(dict(
        x=x, conv_w=conv_w, conv_b=conv_b, w_ih=w_ih, w_hh=w_hh,
        b_ih=b_ih, b_hh=b_hh,
    ))

    res = bass_utils.run_bass_kernel_spmd(nc, in_maps, core_ids=list(range(N_CORES)))
    out = np.concatenate(
        [np.ascontiguousarray(np.asarray(r["out"]).astype(np.float32).T)
         .reshape(B_PER_CORE, L, D)
         for r in res.results], axis=0
    )
    return out


# revision 41
# speedup vs baseline: 1.0134x; 1.0117x over previous
"""LocalRNN Trainium2 kernel: GLU -> pointwise conv -> 9-step windowed LSTM.

Full inputs in, full output out. Sharding: batch across 8 cores (2 batches/core).

Final design (vs the 322us bf16 baseline; measures ~255us, rel err 1.61e-2):
- fp16 pipeline everywhere bf16 was (same size/speed, 10 vs 8 mantissa bits):
  sim rel err 1.4e-3 vs 1.06e-2. The freed error budget pays for fp8.
- LSTM recurrence matmuls for steps 1-7 run in fp8e4 DoubleRow perf mode:
  each MM contracts 256 dims (2 packed/cell) at the same ~220ns FD=512
  cadence as bf16's 128 -> ~1.9x on the h-side PE stream. h written as
  fp8 by the cell's DVE mul; whh pre-packed [128,2,2048] pairs on host.
  Step 8 (error-critical, no forget-gate damping) stays fp16.
  Measured end-to-end rel err 1.611e-2, deterministic across runs.
- PSUM as 4 slots of [128,1024] (one per gate, both batches): a slot's
  fill (PE) and drain (one ACT op) serialize per slot, so big 2048-wide
  tiles bound the unit at fill+drain; 4 small slots pipeline instead.
- Per-gate inject bursts paired (I+G, F+O): batch-dependent gt rotation
  r=(c+q+2b)%4 (two wf copies) gives each 16-matmul burst 16 distinct
  PE (row_grp, col_grp) tiles -> fully concurrent injection.
- ACT order per unit: sigI, tanhG, sigF, sigO, tanhc feeds the DVE chain
  t1=sigI*tanhG, t2=sigF*c, c=t1+t2, h=sigO*tanh(c) without bubbles; each
  unit's tanh(c)+h write is deferred one unit so PSUM drains come earlier.
- Step boundaries: the first DR matmuls of a step wait on the previous
  step's last h8 write (tanh(c) at the end of the saturated ACT stream +
  the DVE fp8 mul). The j==0 unit hoists all four gate injects and every
  contraction-pair-0 MM (h from the early cells j0/j1) ahead of pair-1
  MMs, filling the stall with ~2.7us of runnable PE work so the HAM
  clock gate stops downgrading.
- Step 0 (h=0) skips the F gate entirely.
"""
from contextlib import ExitStack

import numpy as np
import ml_dtypes

import concourse.bass as bass
import concourse.mybir as mybir
import concourse.tile as tile
from concourse import bacc, bass_utils
from concourse.masks import make_identity

F32 = mybir.dt.float32
F16 = mybir.dt.float16
F8 = mybir.dt.float8e4
AF = mybir.ActivationFunctionType
DR = mybir.MatmulPerfMode.DoubleRow

N_CORES = 8
B_PER_CORE = 2          # batches per core
L = 512                 # sequence length
NT = B_PER_CORE * L     # tokens per core = 1024
D = 512                 # model dim
DH = 256                # GLU half dim
G4 = 4 * D              # 2048 gate rows
K = 9                   # window size
PAD = K - 1             # 8
LW = PAD + L            # 520: per-(tile,batch) padded gt row width
TW = B_PER_CORE * LW    # 1040: per-tile width in the big gt tile
S_FP8 = 7               # steps 1..S_FP8 use fp8 DoubleRow; rest fp16

_cache = {}


def _build():
    nc = bacc.Bacc(
        trn_type="TRN2", target_bir_lowering=False, debug=False, num_devices=N_CORES
    )

    xt_d = nc.dram_tensor("xt", [D, NT], F16, kind="ExternalInput").ap()
    wf_d = [nc.dram_tensor(f"wf{b}", [DH, G4], F16, kind="ExternalInput").ap()
            for b in range(2)]
    whh8_d = nc.dram_tensor("whh8", [128, 2 * 2 * G4], F8, kind="ExternalInput").ap()
    whh16_d = nc.dram_tensor("whh16", [D, G4], F16, kind="ExternalInput").ap()
    bias_d = nc.dram_tensor("bias", [128, 64], F32, kind="ExternalInput").ap()
    pad_d = nc.dram_tensor("gtpad", [128, 16 * 2 * PAD], F16, kind="ExternalInput").ap()
    out_d = nc.dram_tensor("out", [D, NT], F16, kind="ExternalOutput").ap()

    with tile.TileContext(nc) as tc, ExitStack() as top:
        const_pool = top.enter_context(tc.tile_pool(name="const", bufs=1))
        w_pool = top.enter_context(tc.tile_pool(name="weights", bufs=1))
        state_pool = top.enter_context(tc.tile_pool(name="state", bufs=1))

        z512 = const_pool.tile([128, 512], F16, tag="z512")
        nc.gpsimd.memset(z512[:], 0.0)
        ident = const_pool.tile([128, 128], F16, tag="id")
        make_identity(nc, ident[:])
        bias_sb = const_pool.tile([128, 64], F32, tag="bias")
        nc.scalar.dma_start(bias_sb[:], bias_d)

        xt_all = w_pool.tile([128, 4 * NT], F16, tag="xt", name="xt")
        xt = [xt_all[:][:, t * NT:(t + 1) * NT] for t in range(4)]
        wf = [[w_pool.tile([128, G4], F16, tag=f"wf{b}_{ck}", name=f"wf{b}_{ck}")
               for ck in range(2)] for b in range(2)]
        whh8 = [w_pool.tile([128, 2 * G4], F8, tag=f"wh8_{p}", name=f"wh8_{p}")
                for p in range(2)]
        whh16 = [w_pool.tile([128, G4], F16, tag=f"wh16_{dk}", name=f"wh16_{dk}")
                 for dk in range(4)]
        # sync queue carries the big loads in need-order; tiny bias/pad on scalar
        for t in (2, 3, 0, 1):
            nc.sync.dma_start(xt[t], xt_d[t * 128:(t + 1) * 128, :])
        for b in range(2):
            for ck in range(2):
                nc.sync.dma_start(wf[b][ck][:], wf_d[b][ck * 128:(ck + 1) * 128, :])
        for p in range(2):
            nc.sync.dma_start(whh8[p][:], whh8_d[:, p * 2 * G4:(p + 1) * 2 * G4])
        for dk in range(4):
            nc.sync.dma_start(whh16[dk][:], whh16_d[dk * 128:(dk + 1) * 128, :])

        # big gt table: 16 gate-row tiles x (2 batches x 520), fp16
        gt = state_pool.tile([128, 16 * TW], F16, tag="gt", name="gt")
        # fp8 h pairs [gen][pair]: [128, 2, NT] -> DoubleRow moving operand
        h8 = [[state_pool.tile([128, 2 * NT], F8, tag=f"h8_{g}_{p}", name=f"h8_{g}_{p}")
               for p in range(2)] for g in range(2)]
        # fp16 h [gen][dk] for steps 7..8
        h16 = [[state_pool.tile([128, NT], F16, tag=f"h16_{g}_{j}", name=f"h16_{g}_{j}")
                for j in range(4)] for g in range(2)]
        cTall = state_pool.tile([128, 4 * NT], F16, tag="c", name="cAll")
        cT = [cTall[:][:, j * NT:(j + 1) * NT] for j in range(4)]
        outT = [state_pool.tile([128, NT], F16, tag=f"o{j}", name=f"o{j}") for j in range(4)]
        uT_all = state_pool.tile([128, 2 * NT], F16, tag="uT", name="uT")
        uT = [uT_all[:][:, ci * NT:(ci + 1) * NT] for ci in range(2)]

        h8v = [[h8[g][p][:].rearrange("p (k n) -> p k n", k=2) for p in range(2)]
               for g in range(2)]
        wh8v = [whh8[p][:].rearrange("p (k m) -> p k m", k=2) for p in range(2)]

        tp = top.enter_context(tc.tile_pool(name="tmp", bufs=4))
        psg = top.enter_context(tc.tile_pool(name="psg", bufs=4, space="PSUM"))

        def warm(n):
            """n dummy N=512 matmul pairs to trip/hold the HAM clock gate."""
            for _ in range(n):
                P = psg.tile([128, 1024], F32, tag="P", name="Pw")
                for r in range(4):
                    nc.tensor.matmul(
                        P[:, (r % 2) * 512:(r % 2) * 512 + 512], ident[:], z512[:],
                        start=True, stop=True,
                    )

        # pad columns (bias-only virtual tokens) land by DMA straight into
        # the strided pad slots of the big gt tile
        pad_view = gt[:].rearrange("p (i b c) -> p i b c", i=16, b=2)[:, :, :, 0:PAD]
        nc.scalar.dma_start(pad_view, pad_d)

        warm(6)  # cold-clock ramp; overlaps input DMA waits

        # GLU on transposed x (one 2048-wide op per engine): xt blocks 2,3
        # hold the sigmoid half contiguously, blocks 0,1 the numerator
        sgt = tp.tile([128, 2 * NT], F16, tag="tT2", name="sgt")
        nc.scalar.activation(sgt[:], xt_all[:][:, 2 * NT:4 * NT], AF.Sigmoid)
        nc.vector.tensor_mul(uT_all[:], xt_all[:][:, 0:2 * NT], sgt[:])

        def g_phase(b, j):
            """gt tiles (q, j) for batch b: conv-folded input gates + bias."""
            for qp in range(2):
                P = psg.tile([128, 1024], F32, tag="P", name="Pg")
                for dq in range(2):
                    q = 2 * qp + dq
                    for ck in range(2):
                        nc.tensor.matmul(
                            P[:, dq * 512:(dq + 1) * 512],
                            wf[b][ck][:, (4 * q + j) * 128:(4 * q + j + 1) * 128],
                            uT[ck][:, b * 512:(b + 1) * 512],
                            start=(ck == 0), stop=(ck == 1),
                        )
                for dq in range(2):
                    q = 2 * qp + dq
                    i = 4 * q + j
                    dst = gt[:, i * TW + b * LW + PAD:i * TW + b * LW + LW]
                    bcol = bias_sb[:, b * 32 + i:b * 32 + i + 1]
                    if q <= 1:
                        nc.vector.tensor_scalar_add(dst, P[:, dq * 512:(dq + 1) * 512], bcol)
                    else:
                        nc.scalar.activation(
                            dst, P[:, dq * 512:(dq + 1) * 512], AF.Identity, bias=bcol,
                        )

        def inject(P, j, q, k, stop):
            """gt -> one [128,1024] gate slot (both batches): 8 32x32 tile MMs.
            Batch-dependent rotation r=(c+q+2b)%4 keeps every paired burst of
            two gates (I+G or F+O) on 16 distinct PE array tiles."""
            for b in range(2):
                off = (4 * q + j) * TW + b * LW + k
                for c in range(4):
                    r = (c + q + 2 * b) % 4
                    nc.tensor.matmul(
                        P[32 * c:32 * c + 32, b * 512:b * 512 + 512],
                        ident[32 * r:32 * r + 32, 32 * r:32 * r + 32],
                        gt[32 * r:32 * r + 32, off:off + 512],
                        start=True, stop=stop,
                        tile_position=(32 * r, 32 * c),
                    )

        def dr8(tiles, j, gr, ponly=None):
            """fp8 DoubleRow recurrence MMs into a list of (P, q) gate slots.
            Contraction-pair-outer order: the pair-0 MMs (h from cells j0/j1,
            written early in the previous step) of every slot run before any
            pair-1 MM needs the last cell's h -> the step-boundary stall on
            the final h8 write shrinks by ~2us of runnable PE work."""
            prange = range(2) if ponly is None else [ponly]
            for p in prange:
                for P, q in tiles:
                    for b in range(2):
                        nc.tensor.matmul(
                            P[:, b * 512:(b + 1) * 512],
                            wh8v[p][:, :, (4 * q + j) * 128:(4 * q + j + 1) * 128],
                            h8v[gr][p][:, :, b * 512:(b + 1) * 512],
                            start=False, stop=(p == 1), perf_mode=DR,
                        )

        def mm16(tiles, j, gr):
            """fp16 recurrence MMs, dk-outer for the same reason as dr8."""
            for dk in range(4):
                for P, q in tiles:
                    for b in range(2):
                        nc.tensor.matmul(
                            P[:, b * 512:(b + 1) * 512],
                            whh16[dk][:, (4 * q + j) * 128:(4 * q + j + 1) * 128],
                            h16[gr][dk][:, b * 512:(b + 1) * 512],
                            start=False, stop=(dk == 3),
                        )

        def cell_front(k, j, PI, PF, PO, PG):
            """gate drains + c update; returns sig(O) for cell_back."""
            first = k == 0
            tI = tp.tile([128, 1024], F16, tag="tI", name="tI")
            nc.scalar.activation(tI[:], PI, AF.Sigmoid)
            tG = tp.tile([128, 1024], F16, tag="tG", name="tG")
            nc.scalar.activation(tG[:], PG, AF.Tanh)
            if first:
                nc.vector.tensor_mul(cT[j], tI[:], tG[:])
            else:
                t1 = tp.tile([128, 1024], F16, tag="t1", name="t1")
                nc.vector.tensor_mul(t1[:], tI[:], tG[:])
                tF = tp.tile([128, 1024], F16, tag="tF", name="tF")
                nc.scalar.activation(tF[:], PF, AF.Sigmoid)
                t2 = tp.tile([128, 1024], F16, tag="tG", name="t2")
                nc.vector.tensor_mul(t2[:], tF[:], cT[j])
                nc.vector.tensor_add(cT[j], t1[:], t2[:])
            tO = tp.tile([128, 1024], F16, tag="tO", name="tO")
            nc.scalar.activation(tO[:], PO, AF.Sigmoid)
            return tO

        def back_half(k, j, tO, tTc, sl):
            nc.scalar.activation(tTc[:, sl], cT[j][:, sl], AF.Tanh)
            if k == K - 1:
                nc.vector.tensor_mul(outT[j][:, sl], tO[:, sl], tTc[:, sl])
            elif k >= S_FP8:
                nc.vector.tensor_mul(h16[k % 2][j][:, sl], tO[:, sl], tTc[:, sl])
            else:
                dst = h8v[k % 2][j // 2][:, j % 2, sl]
                nc.vector.tensor_mul(dst, tO[:, sl], tTc[:, sl])

        def cell_back(k, j, tO):
            """tanh(c) + h write, deferred one unit so the next unit's PSUM
            drains run earlier on ACT."""
            tTc = tp.tile([128, 1024], F16, tag="tTc", name="tTc")
            back_half(k, j, tO, tTc, slice(0, 1024))
            if k == K - 1:
                eng = nc.sync if j % 2 == 0 else nc.scalar
                eng.dma_start(out_d[j * 128:(j + 1) * 128, :], outT[j][:])

        def cell_back_pair(k, tO0, tO1):
            """merged deferred tails of units j0+j1: one 2048-wide tanh over
            the contiguous c slices saves an ACT op overhead per step."""
            tT2 = tp.tile([128, 2048], F16, tag="tT2", name="tT2")
            nc.scalar.activation(tT2[:], cTall[:][:, 0:2 * NT], AF.Tanh)
            for j, tO in ((0, tO0), (1, tO1)):
                sl2 = slice(j * 1024, (j + 1) * 1024)
                if k == K - 1:
                    nc.vector.tensor_mul(outT[j][:], tO[:], tT2[:, sl2])
                    eng = nc.sync if j % 2 == 0 else nc.scalar
                    eng.dma_start(out_d[j * 128:(j + 1) * 128, :], outT[j][:])
                elif k >= S_FP8:
                    nc.vector.tensor_mul(h16[k % 2][j][:], tO[:], tT2[:, sl2])
                else:
                    dst = h8v[k % 2][j // 2][:, j % 2, :]
                    nc.vector.tensor_mul(dst, tO[:], tT2[:, sl2])

        def cell_back_split(k, j, tO):
            """step's last unit: per-batch halves so the first half of the
            next step's h dependency resolves earlier."""
            tTc = tp.tile([128, 1024], F16, tag="tTc", name="tTc")
            back_half(k, j, tO, tTc, slice(0, 512))
            back_half(k, j, tO, tTc, slice(512, 1024))
            if k == K - 1:
                eng = nc.sync if j % 2 == 0 else nc.scalar
                eng.dma_start(out_d[j * 128:(j + 1) * 128, :], outT[j][:])

        def unit0(j):
            """step 0 (h=0): no recurrence, no F gate."""
            PI = psg.tile([128, 1024], F32, tag="P", name="P0I")
            PG = psg.tile([128, 1024], F32, tag="P", name="P0G")
            inject(PI, j, 0, 0, stop=True)
            inject(PG, j, 3, 0, stop=True)
            PO = psg.tile([128, 1024], F32, tag="P", name="P0O")
            inject(PO, j, 2, 0, stop=True)
            return cell_front(0, j, PI[:], None, PO[:], PG[:])

        def unit(k, j):
            gr = (k + 1) % 2
            mms = dr8 if k <= S_FP8 else mm16
            PI = psg.tile([128, 1024], F32, tag="P", name="PI")
            PG = psg.tile([128, 1024], F32, tag="P", name="PG")
            inject(PI, j, 0, k, stop=False)
            inject(PG, j, 3, k, stop=False)
            if j == 0 and k <= S_FP8:
                # step boundary: PE idles ~2.5us on the last h8 write (pair 1).
                # Hoist the F/O injects and every pair-0 MM ahead of the
                # pair-1 MMs: ~2.7us of h8-independent work fills the stall.
                # ACT's first drain waits on the pair-1 stop either way.
                dr8([(PI, 0), (PG, 3)], j, gr, ponly=0)
                PF = psg.tile([128, 1024], F32, tag="P", name="PF")
                PO = psg.tile([128, 1024], F32, tag="P", name="PO")
                inject(PF, j, 1, k, stop=False)
                inject(PO, j, 2, k, stop=False)
                dr8([(PF, 1), (PO, 2)], j, gr, ponly=0)
                dr8([(PI, 0), (PG, 3)], j, gr, ponly=1)
                dr8([(PF, 1), (PO, 2)], j, gr, ponly=1)
            else:
                mms([(PI, 0), (PG, 3)], j, gr)
                PF = psg.tile([128, 1024], F32, tag="P", name="PF")
                PO = psg.tile([128, 1024], F32, tag="P", name="PO")
                inject(PF, j, 1, k, stop=False)
                inject(PO, j, 2, k, stop=False)
                mms([(PF, 1), (PO, 2)], j, gr)
            return cell_front(k, j, PI[:], PF[:], PO[:], PG[:])

        # prep: per-j g_phase for both batches, then step-0 cell;
        # j0+j1 step-0 tails merge like the steady-state pair
        tO0s = []
        for j in range(4):
            g_phase(0, j)
            g_phase(1, j)
            tO0s.append(unit0(j))
            if j == 1:
                cell_back_pair(0, tO0s[0], tO0s[1])
            elif j >= 2:
                cell_back(0, j, tO0s[j])
        warm(2)  # filler across the prep -> steady-state transition

        # ---------------- LSTM steps 1..8 ----------------
        for k in range(1, K):
            tOs = []
            for j in range(4):
                tOs.append(unit(k, j))
                if j == 2:
                    cell_back_pair(k, tOs[0], tOs[1])
            cell_back(k, 2, tOs[2])
            if k < K - 1:
                cell_back_split(k, 3, tOs[3])
            else:
                cell_back(k, 3, tOs[3])

    nc.compile()
    return nc


def _make_in_maps(inputs):
    x = np.asarray(inputs["x"], dtype=np.float32)
    conv_w = np.asarray(inputs["conv_w"], dtype=np.float64)
    conv_b = np.asarray(inputs["conv_b"], dtype=np.float64)
    w_ih = np.asarray(inputs["w_ih"], dtype=np.float64)
    w_hh = np.asarray(inputs["w_hh"], dtype=np.float32)
    b_ih = np.asarray(inputs["b_ih"], dtype=np.float64)
    b_hh = np.asarray(inputs["b_hh"], dtype=np.float64)

    # gate permutation: torch order i,f,g,o -> i,f,o,g
    perm = np.concatenate([
        np.arange(0, D), np.arange(D, 2 * D),
        np.arange(3 * D, 4 * D), np.arange(2 * D, 3 * D),
    ])
    wf_p = (w_ih @ conv_w)[perm]                                # [2048, 256]
    bias_mm = (b_ih + b_hh + w_ih @ conv_b)[perm]
    bias_pad = (b_ih + b_hh)[perm]

    # batch-dependent rotation: tile i (=4q+j) stores logical 32-row block c
    # at partition block r=(c+q+2b)%4 for batch b's gt columns
    def make_rot(b):
        rot = np.empty(G4, dtype=np.int64)
        for i in range(16):
            q = i // 4
            for c in range(4):
                r = (c + q + 2 * b) % 4
                rot[i * 128 + 32 * r: i * 128 + 32 * r + 32] = \
                    np.arange(i * 128 + 32 * c, i * 128 + 32 * c + 32)
        return rot

    shared = {}
    bias_both = np.empty((128, 64), np.float32)
    gtpad = np.empty((128, 16, 2, PAD), np.float16)
    for b in range(2):
        rot = make_rot(b)
        shared[f"wf{b}"] = np.ascontiguousarray(
            wf_p[rot].T.astype(np.float16))                     # [256, 2048]
        bias_both[:, b * 32:b * 32 + 16] = \
            bias_mm[rot].astype(np.float32).reshape(16, 128).T
        bias_both[:, b * 32 + 16:b * 32 + 32] = \
            bias_pad[rot].astype(np.float32).reshape(16, 128).T
        bp = bias_pad[rot].astype(np.float16).reshape(16, 128).T      # [128,16]
        gtpad[:, :, b, :] = bp[:, :, None]
    shared["bias"] = bias_both
    shared["gtpad"] = np.ascontiguousarray(gtpad.reshape(128, -1))

    whhT = np.ascontiguousarray(w_hh[perm].T)                   # [512, 2048]
    shared["whh16"] = whhT.astype(np.float16)
    blocks = whhT.reshape(4, 128, G4)
    pairs = np.stack([np.stack([blocks[2 * p], blocks[2 * p + 1]], axis=1)
                      for p in range(2)])                       # [2, 128, 2, 2048]
    shared["whh8"] = np.ascontiguousarray(
        pairs.transpose(1, 0, 2, 3).reshape(128, -1).astype(ml_dtypes.float8_e4m3fn))

    in_maps = []
    for c in range(N_CORES):
        m = dict(shared)
        xc = x[c * B_PER_CORE:(c + 1) * B_PER_CORE].reshape(NT, D)
        m["xt"] = np.ascontiguousarray(xc.T.astype(np.float16))  # [512, 1024]
        in_maps.append(m)
    return in_maps


def kernel(x, conv_w, conv_b, w_ih, w_hh, b_ih, b_hh):
    if "nc" not in _cache:
        _cache["nc"] = _build()
    nc = _cache["nc"]

    in_maps = _make_in_maps(dict(
        x=x, conv_w=conv_w, conv_b=conv_b, w_ih=w_ih, w_hh=w_hh,
        b_ih=b_ih, b_hh=b_hh,
    ))

    res = bass_utils.run_bass_kernel_spmd(nc, in_maps, core_ids=list(range(N_CORES)))
    out = np.concatenate(
        [np.ascontiguousarray(np.asarray(r["out"]).astype(np.float32).T)
         .reshape(B_PER_CORE, L, D)
         for r in res.results], axis=0
    )
    return out
